# revision 1
# baseline (speedup 1.0000x reference)
"""Contextual attention kernel for Trainium2 (8 NeuronCores, data-parallel over batch).

Math (per batch b):
    Q = feaQK @ q_w.T + q_b
    k3 = conv1d(feaQK.T, cn3_w, SAME) + b3 ; k5 = conv1d(..., cn5_w) + b5
    K = [feaQK, k3, k5] @ k_w.T + k_b
    V = feaV @ v_w.T + v_b
    S = (Q @ K.T) / sqrt(D); mask keys >= seqlen with -inf
    out = softmax(S) @ V + V

Kernel strategy:
  * The convs + concat + K-projection collapse into a single width-5 stencil:
        K[s] = sum_{d=-2..2} feaQK[s+d] @ Wk[d] + kb_eff
    with Wk composed on the host (15 matmul-units of work -> 9).
  * All activations live on-chip in transposed layout ([feature, seq]) so no
    on-device transposes are needed anywhere:
        QT/KT from xT (host-transposed feaQK, zero-padded +-2 cols)
        scoresT[k,q] = KT chunks (stationary) x QT  (PSUM fp32)
        ET = exp(scoresT/32 + mask_bias[k])  (mask folded into exp bias; no
             max-subtraction needed since |scores/32| is O(1))
        V (natural [s,d]) from host-transposed feaV as the stationary operand
        outU[q,d] = ET chunks (stationary) x V; den[q] = ET x ones
        out = outU / den + V
  * Matmuls in bf16 (fp32 matmul is 4x slower on PE), fp32 PSUM accumulation.
  * Keys beyond seqlength are dead: K/scores/PV work only covers the first
    ceil(seqlen/128) key chunks per batch slot. Batches are paired
    longest-with-shortest across cores so the compile-time per-slot chunk
    counts (max over cores) stay small; sub-chunk masking still goes through
    the exp bias, so over-covering is always correct.
  * 16 batches -> 2 per core, full weights on every core.
"""

import numpy as np
import ml_dtypes

import concourse.bass as bass
from concourse import bacc
import concourse.tile as tile
from concourse import mybir

B, S, C, D = 16, 1024, 1024, 1024
P = 128
NCI, NDI, NKI, NQI, NSI = C // P, D // P, S // P, S // P, S // P
NF = 512  # matmul free dim (one PSUM bank of fp32)
PAD = 2
SP = S + 2 * PAD
LB = 2  # local batches per core
NCORES = 8
MASK_NEG = -60000.0
SCALE = 1.0 / 32.0  # 1/sqrt(D)

BF = mybir.dt.bfloat16
F32 = mybir.dt.float32
AF = mybir.ActivationFunctionType

TRACE = False  # set by test harness to collect HW profile
_CACHE = {}


def _build_program(vs):
    nc = bacc.Bacc("TRN2", dynamic_dma_scratch_size=256)

    xt = nc.dram_tensor("xt", [LB, C, SP], BF, kind="ExternalInput")
    fvt = nc.dram_tensor("fvt", [LB, C, S], BF, kind="ExternalInput")
    wq = nc.dram_tensor("wq", [C, D], BF, kind="ExternalInput")
    wk = nc.dram_tensor("wk", [5, C, D], BF, kind="ExternalInput")
    wv = nc.dram_tensor("wv", [C, D], BF, kind="ExternalInput")
    qb = nc.dram_tensor("qb", [P, NDI], F32, kind="ExternalInput")
    kb = nc.dram_tensor("kb", [P, NDI], F32, kind="ExternalInput")
    vb = nc.dram_tensor("vb", [P, D], F32, kind="ExternalInput")
    mb = nc.dram_tensor("mb", [LB, P, NKI], F32, kind="ExternalInput")
    out = nc.dram_tensor("out", [LB, S, D], F32, kind="ExternalOutput")

    with tile.TileContext(nc) as tc:
        _emit(nc, tc, xt, fvt, wq, wk, wv, qb, kb, vb, mb, out, vs)
    nc.finalize()
    return nc


def _emit(nc, tc, xt, fvt, wq, wk, wv, qb, kb, vb, mb, out, vs):
    from contextlib import ExitStack

    with ExitStack() as ctx:
        wpool = ctx.enter_context(tc.tile_pool(name="wpool", bufs=1))
        apool = ctx.enter_context(tc.tile_pool(name="apool", bufs=1))
        opool = ctx.enter_context(tc.tile_pool(name="opool", bufs=3))
        spool = ctx.enter_context(tc.tile_pool(name="spool", bufs=2))
        pp = ctx.enter_context(tc.tile_pool(name="pp", bufs=6, space="PSUM"))
        pd = ctx.enter_context(tc.tile_pool(name="pd", bufs=2, space="PSUM"))

        # Small constants first (cheap), then per-stage operands in the order
        # the PE consumes them, so the first matmul isn't stuck behind the
        # whole 18 MiB initial load (measured 51 us of PE idle).
        QB = wpool.tile([P, NDI], F32, tag="qb")
        nc.sync.dma_start(out=QB, in_=qb[:, :])
        KB = wpool.tile([P, NDI], F32, tag="kb")
        nc.sync.dma_start(out=KB, in_=kb[:, :])
        VB = wpool.tile([P, D], F32, tag="vb")
        nc.sync.dma_start(out=VB, in_=vb[:, :])
        ONES = wpool.tile([P, 1], BF, tag="ones")
        nc.vector.memset(ONES, 1.0)
        WV = wpool.tile([P, NCI, D], BF, tag="wv")
        WQ = wpool.tile([P, NCI, D], BF, tag="wq")
        WK = None

        for b in range(LB):
            v = vs[b]  # valid key chunks for this batch slot
            # key-dim psum groups: (offset, width) pieces covering v*128 cols
            kg = [(0, min(v * P, NF))]
            if v * P > NF:
                kg.append((NF, v * P - NF))

            # --- stage D: V natural [s, d] (first: smallest DMA prefix) --
            FVT = apool.tile([P, NCI, S], BF, tag="fvt")
            for ci in range(NCI):
                nc.sync.dma_start(out=FVT[:, ci, :], in_=fvt[b, ci * P:(ci + 1) * P, :])
                if b == 0:
                    nc.sync.dma_start(out=WV[:, ci, :], in_=wv[ci * P:(ci + 1) * P, :])
            V = apool.tile([P, NSI, D], BF, tag="v")
            for si in range(NSI):
                ps = [pp.tile([P, NF], F32, tag="ps", name=f"ps{_i}") for _i in range(2)]
                for ci in range(NCI):
                    lhsT = FVT[:, ci, si * P:(si + 1) * P]
                    for dh in range(2):
                        nc.tensor.matmul(
                            ps[dh], lhsT, WV[:, ci, dh * NF:(dh + 1) * NF],
                            start=(ci == 0), stop=(ci == NCI - 1))
                for dh in range(2):
                    nc.vector.tensor_add(
                        V[:, si, dh * NF:(dh + 1) * NF], ps[dh],
                        VB[:, dh * NF:(dh + 1) * NF])

            # --- stage B: QT[d, s] ---------------------------------------
            XT = apool.tile([P, NCI, SP], BF, tag="xt")
            for ci in range(NCI):
                nc.sync.dma_start(out=XT[:, ci, :], in_=xt[b, ci * P:(ci + 1) * P, :])
                if b == 0:
                    nc.sync.dma_start(out=WQ[:, ci, :], in_=wq[ci * P:(ci + 1) * P, :])
            MB = spool.tile([P, NKI], F32, tag="mb")
            nc.sync.dma_start(out=MB, in_=mb[b])
            QT = apool.tile([P, NDI, S], BF, tag="qt")
            for di in range(NDI):
                ps = [pp.tile([P, NF], F32, tag="ps", name=f"ps{_i}") for _i in range(2)]
                for ci in range(NCI):
                    lhsT = WQ[:, ci, di * P:(di + 1) * P]
                    for sh in range(2):
                        nc.tensor.matmul(
                            ps[sh], lhsT, XT[:, ci, PAD + sh * NF: PAD + sh * NF + NF],
                            start=(ci == 0), stop=(ci == NCI - 1))
                for sh in range(2):
                    nc.scalar.activation(
                        QT[:, di, sh * NF:(sh + 1) * NF], ps[sh], AF.Identity,
                        bias=QB[:, di:di + 1], scale=1.0)

            # --- stage C: KT[d, s] (width-5 stencil, only v key chunks) --
            if WK is None:
                WK = []
                for j in range(5):
                    t = wpool.tile([P, NCI, D], BF, tag=f"wk{j}")
                    for ci in range(NCI):
                        nc.sync.dma_start(
                            out=t[:, ci, :], in_=wk[j, ci * P:(ci + 1) * P, :])
                    WK.append(t)
            KT = apool.tile([P, NDI, S], BF, tag="kt")
            for di in range(NDI):
                ps = [pp.tile([P, NF], F32, tag="ps", name=f"ps{_i}")
                      for _i in range(len(kg))]
                step = 0
                for j in range(5):
                    for ci in range(NCI):
                        lhsT = WK[j][:, ci, di * P:(di + 1) * P]
                        for g, (off, w) in enumerate(kg):
                            nc.tensor.matmul(
                                ps[g][:, :w], lhsT,
                                XT[:, ci, j + off: j + off + w],
                                start=(step == 0), stop=(step == 5 * NCI - 1))
                        step += 1
                for g, (off, w) in enumerate(kg):
                    nc.scalar.activation(
                        KT[:, di, off:off + w], ps[g][:, :w], AF.Identity,
                        bias=KB[:, di:di + 1], scale=1.0)

            # --- stage E: ET[k, q] = exp(scoresT/32 + mask) --------------
            ET = apool.tile([P, NKI, S], BF, tag="et")
            for ki in range(v):
                ps = [pp.tile([P, NF], F32, tag="ps", name=f"ps{_i}") for _i in range(2)]
                for di in range(NDI):
                    lhsT = KT[:, di, ki * P:(ki + 1) * P]
                    for qh in range(2):
                        nc.tensor.matmul(
                            ps[qh], lhsT, QT[:, di, qh * NF:(qh + 1) * NF],
                            start=(di == 0), stop=(di == NDI - 1))
                for qh in range(2):
                    nc.scalar.activation(
                        ET[:, ki, qh * NF:(qh + 1) * NF], ps[qh], AF.Exp,
                        bias=MB[:, ki:ki + 1], scale=SCALE)

            # --- stage F: out = (ET^T @ V) / den + V ---------------------
            for qi in range(NQI):
                pso = [pp.tile([P, NF], F32, tag="ps", name=f"pso{_i}") for _i in range(2)]
                psd = pd.tile([P, 1], F32, tag="den")
                for ki in range(v):
                    lhsT = ET[:, ki, qi * P:(qi + 1) * P]
                    st, sp_ = (ki == 0), (ki == v - 1)
                    for dh in range(2):
                        nc.tensor.matmul(
                            pso[dh], lhsT, V[:, ki, dh * NF:(dh + 1) * NF],
                            start=st, stop=sp_)
                    nc.tensor.matmul(psd, lhsT, ONES, start=st, stop=sp_)
                # Free the PSUM banks with plain DVE copies that wait only on
                # the matmul stop; the reciprocal-scale and +V run in place on
                # SBUF afterwards, off the PE-critical path.
                OTs = []
                for dh in range(2):
                    OT = opool.tile([P, NF], F32, tag="out", name=f"ot{dh}")
                    nc.vector.tensor_copy(OT, pso[dh])
                    OTs.append(OT)
                REC = spool.tile([P, 1], F32, tag="rec")
                nc.vector.reciprocal(REC, psd)
                for dh in range(2):
                    OT = OTs[dh]
                    nc.scalar.activation(
                        OT, OT, AF.Copy, bias=0.0, scale=REC)
                    nc.vector.tensor_add(
                        OT, OT, V[:, qi, dh * NF:(dh + 1) * NF])
                    nc.sync.dma_start(
                        out=out[b, qi * P:(qi + 1) * P, dh * NF:(dh + 1) * NF],
                        in_=OT)


def _prep_host(feaQK, feaV, seqlengths, cn3_w, cn3_b, cn5_w, cn5_b,
               k_w, k_b, q_w, q_b, v_w, v_b):
    """Compose weights, assign batches to cores, lay out per-core inputs."""
    f32 = np.float32
    bf16 = ml_dtypes.bfloat16
    feaQK = np.asarray(feaQK, f32)
    feaV = np.asarray(feaV, f32)
    seqlengths = np.asarray(seqlengths).astype(np.int64)

    W1 = np.asarray(k_w, f32)[:, :C]
    W2 = np.asarray(k_w, f32)[:, C:2 * C]
    W3 = np.asarray(k_w, f32)[:, 2 * C:]

    wk = np.zeros((5, C, D), f32)  # [tap j (= shift+2), c, d]
    for t in range(3):
        wk[t + 1] += (W2 @ np.asarray(cn3_w, f32)[:, :, t]).T
    for t in range(5):
        wk[t] += (W3 @ np.asarray(cn5_w, f32)[:, :, t]).T
    wk[2] += W1.T
    kb_eff = (np.asarray(k_b, f32) + W2 @ np.asarray(cn3_b, f32)
              + W3 @ np.asarray(cn5_b, f32))

    wq = np.ascontiguousarray(np.asarray(q_w, f32).T)
    wv = np.ascontiguousarray(np.asarray(v_w, f32).T)

    qb_pd = np.ascontiguousarray(np.asarray(q_b, f32).reshape(NDI, P).T)
    kb_pd = np.ascontiguousarray(kb_eff.reshape(NDI, P).T)
    vb_rep = np.ascontiguousarray(
        np.broadcast_to(np.asarray(v_b, f32), (P, D)))

    key_valid = np.arange(S)[None, :] < seqlengths[:, None]
    mask = np.where(key_valid, 0.0, MASK_NEG).astype(f32)  # [B, S]

    # Pair longest with shortest so the compile-time per-slot chunk counts
    # (max over cores) stay near the per-core optimum.
    vchunks = np.clip(np.ceil(seqlengths / P).astype(int), 1, NKI)
    order = np.argsort(-seqlengths, kind="stable")
    batch_of = np.zeros((NCORES, LB), int)
    for i in range(NCORES):
        batch_of[i, 0] = order[B - 1 - i]
        batch_of[i, 1] = order[i]
    vs = (int(vchunks[batch_of[:, 0]].max()),
          int(vchunks[batch_of[:, 1]].max()))

    wq_b = wq.astype(bf16)
    wk_b = np.ascontiguousarray(wk.astype(bf16))
    wv_b = wv.astype(bf16)

    in_maps = []
    for core in range(NCORES):
        bs = batch_of[core]
        xts = np.zeros((LB, C, SP), bf16)
        xts[:, :, PAD:PAD + S] = feaQK[bs].transpose(0, 2, 1).astype(bf16)
        fvts = np.ascontiguousarray(
            feaV[bs].transpose(0, 2, 1)).astype(bf16)
        mbs = np.ascontiguousarray(
            mask[bs].reshape(LB, NKI, P).transpose(0, 2, 1))
        in_maps.append({
            "xt": xts, "fvt": fvts,
            "wq": wq_b, "wk": wk_b, "wv": wv_b,
            "qb": qb_pd, "kb": kb_pd, "vb": vb_rep, "mb": mbs,
        })
    return in_maps, batch_of, vs


def kernel(**inputs):
    from concourse.bass_utils import run_bass_kernel_spmd

    in_maps, batch_of, vs = _prep_host(**inputs)
    if _CACHE.get("vs") != vs:
        _CACHE["nc"] = _build_program(vs)
        _CACHE["vs"] = vs
    nc = _CACHE["nc"]
    res = run_bass_kernel_spmd(nc, in_maps, core_ids=list(range(NCORES)),
                               trace=TRACE)
    _CACHE["last_result"] = res
    full = np.zeros((B, S, D), np.float32)
    for core in range(NCORES):
        full[batch_of[core]] = res.results[core]["out"]
    return full



# revision 2
# speedup vs baseline: 1.6372x; 1.6372x over previous
"""Contextual attention kernel for Trainium2 (8 NeuronCores, data-parallel over batch).

Math (per batch b):
    Q = feaQK @ q_w.T + q_b
    k3 = conv1d(feaQK.T, cn3_w, SAME) + b3 ; k5 = conv1d(..., cn5_w) + b5
    K = [feaQK, k3, k5] @ k_w.T + k_b
    V = feaV @ v_w.T + v_b
    S = (Q @ K.T) / sqrt(D); mask keys >= seqlen with -inf
    out = softmax(S) @ V + V

Kernel strategy:
  * The convs + concat + K-projection collapse into a single width-5 stencil:
        K[s] = sum_{d=-2..2} feaQK[s+d] @ Wk[d] + kb_eff
    with Wk composed on the host (15 matmul-units of work -> 9).
  * All activations live on-chip in transposed layout ([feature, seq]) so no
    on-device transposes are needed anywhere:
        QT/KT from xT (host-transposed feaQK, zero-padded cols)
        scoresT[k,q] = KT chunks (stationary) x QT  (PSUM fp32)
        ET16 = 16*exp(scoresT/32 + mask)  (mask folded into exp bias)
        V0 (natural [s,d]) from host-transposed feaV as the stationary operand
        outU[q,d] = ET16 chunks (stationary) x V8; den[q] = ET16 x ones
        out = outU / den + (V0 + 2*vb)        [atten@vb == vb since sum(atten)=1]
  * fp8e4 DoubleRow matmuls (2 contraction blocks / instruction, ~1.5-1.8x bf16)
    for the Q-proj, K-stencil, scores and PV stages. Weights are scaled x256 on
    the host so they sit in fp8's normal range; Q/K are stored x16; ET x16.
    The V-projection stays bf16: out ~= V, so V's accuracy dominates the
    output and fp8 would blow the error budget. The PV matmul uses a separate
    fp8 copy (V8) of the unbiased projection.
  * Keys beyond seqlength are dead: K/scores/PV work only covers the first
    ceil(seqlen/128) key chunks per batch slot. Batches are paired
    longest-with-shortest across cores so the compile-time per-slot chunk
    counts (max over cores) stay small; sub-chunk masking still goes through
    the exp bias, so over-covering is always correct.
  * 16 batches -> 2 per core, full weights on every core. Output DMA in bf16.
"""

import numpy as np
import ml_dtypes

import concourse.bass as bass
from concourse import bacc
import concourse.tile as tile
from concourse import mybir

B, S, C, D = 16, 1024, 1024, 1024
P = 128
NCI, NDI, NKI, NQI, NSI = C // P, D // P, S // P, S // P, S // P
NF = 512  # matmul free dim (one PSUM bank of fp32)
PADL = 2   # left zero pad for the width-5 stencil
SP = 1040  # padded seq width; multiple of 16 so fp8 DoubleRow pair-stride is legal
LB = 2  # local batches per core
NCORES = 8
MASK_NEG = -60000.0
SCALE = 1.0 / 32.0  # 1/sqrt(D)
WS = 256.0   # host weight scale into fp8 normal range
AS = 16.0    # on-chip activation scale for QT/KT/ET
LOG_AS = float(np.log(AS))

BF = mybir.dt.bfloat16
F32 = mybir.dt.float32
F8 = mybir.dt.float8e4
AF = mybir.ActivationFunctionType
DR = mybir.MatmulPerfMode.DoubleRow

TRACE = False  # set by test harness to collect HW profile
_CACHE = {}


def _build_program(vs):
    nc = bacc.Bacc("TRN2", dynamic_dma_scratch_size=256)

    xt = nc.dram_tensor("xt", [LB, C, SP], F8, kind="ExternalInput")
    fvt = nc.dram_tensor("fvt", [LB, C, S], BF, kind="ExternalInput")
    wq = nc.dram_tensor("wq", [C, D], F8, kind="ExternalInput")
    wk = nc.dram_tensor("wk", [5, C, D], F8, kind="ExternalInput")
    wv = nc.dram_tensor("wv", [C, D], BF, kind="ExternalInput")
    qb = nc.dram_tensor("qb", [P, NDI], F32, kind="ExternalInput")
    kb = nc.dram_tensor("kb", [P, NDI], F32, kind="ExternalInput")
    vb = nc.dram_tensor("vb", [P, D], F32, kind="ExternalInput")
    mb = nc.dram_tensor("mb", [LB, P, NKI], F32, kind="ExternalInput")
    out = nc.dram_tensor("out", [LB, S, D], BF, kind="ExternalOutput")

    with tile.TileContext(nc) as tc:
        _emit(nc, tc, xt, fvt, wq, wk, wv, qb, kb, vb, mb, out, vs)
    nc.finalize()
    return nc


def _emit(nc, tc, xt, fvt, wq, wk, wv, qb, kb, vb, mb, out, vs):
    from contextlib import ExitStack

    with ExitStack() as ctx:
        wpool = ctx.enter_context(tc.tile_pool(name="wpool", bufs=1))
        apool = ctx.enter_context(tc.tile_pool(name="apool", bufs=1))
        opool = ctx.enter_context(tc.tile_pool(name="opool", bufs=3))
        spool = ctx.enter_context(tc.tile_pool(name="spool", bufs=2))
        pp = ctx.enter_context(tc.tile_pool(name="pp", bufs=6, space="PSUM"))
        pd = ctx.enter_context(tc.tile_pool(name="pd", bufs=2, space="PSUM"))

        # Small constants first (cheap), then per-stage operands in the order
        # the PE consumes them, so the first matmul isn't stuck behind the
        # whole initial weight load.
        QB = wpool.tile([P, NDI], F32, tag="qb")
        nc.sync.dma_start(out=QB, in_=qb[:, :])
        KB = wpool.tile([P, NDI], F32, tag="kb")
        nc.sync.dma_start(out=KB, in_=kb[:, :])
        VB2 = wpool.tile([P, D], F32, tag="vb")
        nc.sync.dma_start(out=VB2, in_=vb[:, :])
        ONES = wpool.tile([P, 2, 16], F8, tag="ones")
        nc.vector.memset(ONES, 1.0)
        WV = wpool.tile([P, NCI, D], BF, tag="wv")
        WQ = wpool.tile([P, NCI, D], F8, tag="wq")
        WK = None

        for b in range(LB):
            v = vs[b]  # valid key chunks for this batch slot
            # key-dim psum groups: (offset, width) pieces covering v*128 cols
            kg = [(0, min(v * P, NF))]
            if v * P > NF:
                kg.append((NF, v * P - NF))

            # --- stage B: QT[d, s] = 16*Q (fp8 DoubleRow; smallest DMA lead-in)
            XT = apool.tile([P, NCI, SP], F8, tag="xt")
            for ci in range(NCI):
                nc.sync.dma_start(out=XT[:, ci, :], in_=xt[b, ci * P:(ci + 1) * P, :])
                if b == 0:
                    nc.sync.dma_start(out=WQ[:, ci, :], in_=wq[ci * P:(ci + 1) * P, :])
            MB = spool.tile([P, NKI], F32, tag="mb")
            nc.sync.dma_start(out=MB, in_=mb[b])
            QT = apool.tile([P, NDI, S], F8, tag="qt")
            for di in range(NDI):
                ps = [pp.tile([P, NF], F32, tag="ps", name=f"ps{_i}") for _i in range(2)]
                for cp in range(0, NCI, 2):
                    lhsT = WQ[:, cp:cp + 2, di * P:(di + 1) * P]
                    for sh in range(2):
                        nc.tensor.matmul(
                            ps[sh], lhsT,
                            XT[:, cp:cp + 2, PADL + sh * NF: PADL + sh * NF + NF],
                            start=(cp == 0), stop=(cp == NCI - 2), perf_mode=DR)
                for sh in range(2):
                    nc.scalar.activation(
                        QT[:, di, sh * NF:(sh + 1) * NF], ps[sh], AF.Identity,
                        bias=QB[:, di:di + 1], scale=AS / WS)

            # --- stage D: V0 natural [s, d]; bf16 Vb2 for +V, fp8 V8 for PV --
            FVT = apool.tile([P, NCI, S], BF, tag="fvt")
            for ci in range(NCI):
                nc.sync.dma_start(out=FVT[:, ci, :], in_=fvt[b, ci * P:(ci + 1) * P, :])
                if b == 0:
                    nc.sync.dma_start(out=WV[:, ci, :], in_=wv[ci * P:(ci + 1) * P, :])
            V = apool.tile([P, NSI, D], BF, tag="v")
            V8 = apool.tile([P, NSI, D], F8, tag="v8")
            for si in range(NSI):
                ps = [pp.tile([P, NF], F32, tag="ps", name=f"ps{_i}") for _i in range(2)]
                for ci in range(NCI):
                    lhsT = FVT[:, ci, si * P:(si + 1) * P]
                    for dh in range(2):
                        nc.tensor.matmul(
                            ps[dh], lhsT, WV[:, ci, dh * NF:(dh + 1) * NF],
                            start=(ci == 0), stop=(ci == NCI - 1))
                for dh in range(2):
                    nc.vector.tensor_add(
                        V[:, si, dh * NF:(dh + 1) * NF], ps[dh],
                        VB2[:, dh * NF:(dh + 1) * NF])
                    if si < v:
                        nc.vector.tensor_copy(
                            V8[:, si, dh * NF:(dh + 1) * NF], ps[dh])

            # --- stage C: KT[d, s] = 16*K (width-5 stencil, only v key chunks)
            if WK is None:
                WK = []
                for j in range(5):
                    t = wpool.tile([P, NCI, D], F8, tag=f"wk{j}")
                    for ci in range(NCI):
                        nc.sync.dma_start(
                            out=t[:, ci, :], in_=wk[j, ci * P:(ci + 1) * P, :])
                    WK.append(t)
            KT = apool.tile([P, NDI, S], F8, tag="kt")
            for di in range(NDI):
                ps = [pp.tile([P, NF], F32, tag="ps", name=f"ps{_i}")
                      for _i in range(len(kg))]
                step, nsteps = 0, 5 * (NCI // 2)
                for j in range(5):
                    for cp in range(0, NCI, 2):
                        lhsT = WK[j][:, cp:cp + 2, di * P:(di + 1) * P]
                        for g, (off, w) in enumerate(kg):
                            nc.tensor.matmul(
                                ps[g][:, :w], lhsT,
                                XT[:, cp:cp + 2, j + off: j + off + w],
                                start=(step == 0), stop=(step == nsteps - 1),
                                perf_mode=DR)
                        step += 1
                for g, (off, w) in enumerate(kg):
                    nc.scalar.activation(
                        KT[:, di, off:off + w], ps[g][:, :w], AF.Identity,
                        bias=KB[:, di:di + 1], scale=AS / WS)

            # --- stage E: ET16[k, q] = 16*exp(scoresT/32 + mask) -------------
            ET = apool.tile([P, NKI, S], F8, tag="et")
            for ki in range(v):
                ps = [pp.tile([P, NF], F32, tag="ps", name=f"ps{_i}") for _i in range(2)]
                for dp in range(0, NDI, 2):
                    lhsT = KT[:, dp:dp + 2, ki * P:(ki + 1) * P]
                    for qh in range(2):
                        nc.tensor.matmul(
                            ps[qh], lhsT, QT[:, dp:dp + 2, qh * NF:(qh + 1) * NF],
                            start=(dp == 0), stop=(dp == NDI - 2), perf_mode=DR)
                for qh in range(2):
                    nc.scalar.activation(
                        ET[:, ki, qh * NF:(qh + 1) * NF], ps[qh], AF.Exp,
                        bias=MB[:, ki:ki + 1], scale=SCALE / (AS * AS))

            # --- stage F: out = (ET16^T @ V8) / den + Vb2 --------------------
            for qi in range(NQI):
                pso = [pp.tile([P, NF], F32, tag="ps", name=f"pso{_i}") for _i in range(2)]
                psd = pd.tile([P, 1], F32, tag="den")
                for kp in range(0, v - (v % 2), 2):
                    lhsT = ET[:, kp:kp + 2, qi * P:(qi + 1) * P]
                    st, sp_ = (kp == 0), (kp + 2 >= v)
                    for dh in range(2):
                        nc.tensor.matmul(
                            pso[dh], lhsT, V8[:, kp:kp + 2, dh * NF:(dh + 1) * NF],
                            start=st, stop=sp_, perf_mode=DR)
                    nc.tensor.matmul(psd, lhsT, ONES[:, :, 0:1],
                                     start=st, stop=sp_, perf_mode=DR)
                if v % 2:
                    ki = v - 1
                    lhsT = ET[:, ki, qi * P:(qi + 1) * P]
                    st = (v == 1)
                    for dh in range(2):
                        nc.tensor.matmul(
                            pso[dh], lhsT, V8[:, ki, dh * NF:(dh + 1) * NF],
                            start=st, stop=True)
                    nc.tensor.matmul(psd, lhsT, ONES[:, 0, 0:1],
                                     start=st, stop=True)
                REC = spool.tile([P, 1], F32, tag="rec")
                nc.vector.reciprocal(REC, psd)
                for dh in range(2):
                    OT = opool.tile([P, NF], F32, tag="otf", name=f"ot{dh}")
                    nc.scalar.activation(OT, pso[dh], AF.Copy, bias=0.0, scale=REC)
                    OB = opool.tile([P, NF], BF, tag="obf", name=f"ob{dh}")
                    nc.vector.tensor_add(
                        OB, OT, V[:, qi, dh * NF:(dh + 1) * NF])
                    nc.sync.dma_start(
                        out=out[b, qi * P:(qi + 1) * P, dh * NF:(dh + 1) * NF],
                        in_=OB)


def _prep_host(feaQK, feaV, seqlengths, cn3_w, cn3_b, cn5_w, cn5_b,
               k_w, k_b, q_w, q_b, v_w, v_b):
    """Compose weights, assign batches to cores, lay out per-core inputs."""
    f32 = np.float32
    bf16 = ml_dtypes.bfloat16
    f8 = ml_dtypes.float8_e4m3
    feaQK = np.asarray(feaQK, f32)
    feaV = np.asarray(feaV, f32)
    seqlengths = np.asarray(seqlengths).astype(np.int64)

    W1 = np.asarray(k_w, f32)[:, :C]
    W2 = np.asarray(k_w, f32)[:, C:2 * C]
    W3 = np.asarray(k_w, f32)[:, 2 * C:]

    wk = np.zeros((5, C, D), f32)  # [tap j (= shift+2), c, d]
    for t in range(3):
        wk[t + 1] += (W2 @ np.asarray(cn3_w, f32)[:, :, t]).T
    for t in range(5):
        wk[t] += (W3 @ np.asarray(cn5_w, f32)[:, :, t]).T
    wk[2] += W1.T
    kb_eff = (np.asarray(k_b, f32) + W2 @ np.asarray(cn3_b, f32)
              + W3 @ np.asarray(cn5_b, f32))

    wq = np.ascontiguousarray(np.asarray(q_w, f32).T)
    wv = np.ascontiguousarray(np.asarray(v_w, f32).T)

    qb_pd = np.ascontiguousarray((np.asarray(q_b, f32) * AS).reshape(NDI, P).T)
    kb_pd = np.ascontiguousarray((kb_eff * AS).reshape(NDI, P).T)
    vb2_rep = np.ascontiguousarray(
        np.broadcast_to(2.0 * np.asarray(v_b, f32), (P, D)))

    key_valid = np.arange(S)[None, :] < seqlengths[:, None]
    mask = np.where(key_valid, LOG_AS, MASK_NEG).astype(f32)  # [B, S]

    # Pair longest with shortest so the compile-time per-slot chunk counts
    # (max over cores) stay near the per-core optimum.
    vchunks = np.clip(np.ceil(seqlengths / P).astype(int), 1, NKI)
    order = np.argsort(-seqlengths, kind="stable")
    batch_of = np.zeros((NCORES, LB), int)
    for i in range(NCORES):
        batch_of[i, 0] = order[B - 1 - i]
        batch_of[i, 1] = order[i]
    vs = (int(vchunks[batch_of[:, 0]].max()),
          int(vchunks[batch_of[:, 1]].max()))

    wq_8 = np.clip(wq * WS, -240, 240).astype(f8)
    wk_8 = np.ascontiguousarray(np.clip(wk * WS, -240, 240).astype(f8))
    wv_b = wv.astype(bf16)

    in_maps = []
    for core in range(NCORES):
        bs = batch_of[core]
        xts = np.zeros((LB, C, SP), f8)
        xts[:, :, PADL:PADL + S] = np.clip(
            feaQK[bs].transpose(0, 2, 1), -240, 240).astype(f8)
        fvts = np.ascontiguousarray(
            feaV[bs].transpose(0, 2, 1)).astype(bf16)
        mbs = np.ascontiguousarray(
            mask[bs].reshape(LB, NKI, P).transpose(0, 2, 1))
        in_maps.append({
            "xt": xts, "fvt": fvts,
            "wq": wq_8, "wk": wk_8, "wv": wv_b,
            "qb": qb_pd, "kb": kb_pd, "vb": vb2_rep, "mb": mbs,
        })
    return in_maps, batch_of, vs


def kernel(**inputs):
    from concourse.bass_utils import run_bass_kernel_spmd

    in_maps, batch_of, vs = _prep_host(**inputs)
    if _CACHE.get("vs") != vs:
        _CACHE["nc"] = _build_program(vs)
        _CACHE["vs"] = vs
    nc = _CACHE["nc"]
    res = run_bass_kernel_spmd(nc, in_maps, core_ids=list(range(NCORES)),
                               trace=TRACE)
    _CACHE["last_result"] = res
    full = np.zeros((B, S, D), np.float32)
    for core in range(NCORES):
        full[batch_of[core]] = res.results[core]["out"].astype(np.float32)
    return full


# revision 5
# speedup vs baseline: 1.6570x; 1.0121x over previous
"""Contextual attention kernel for Trainium2 (8 NeuronCores, data-parallel over batch).

Math (per batch b):
    Q = feaQK @ q_w.T + q_b
    k3 = conv1d(feaQK.T, cn3_w, SAME) + b3 ; k5 = conv1d(..., cn5_w) + b5
    K = [feaQK, k3, k5] @ k_w.T + k_b
    V = feaV @ v_w.T + v_b
    S = (Q @ K.T) / sqrt(D); mask keys >= seqlen with -inf
    out = softmax(S) @ V + V

Kernel strategy:
  * The convs + concat + K-projection collapse into a single width-5 stencil:
        K[s] = sum_{d=-2..2} feaQK[s+d] @ Wk[d] + kb_eff
    with Wk composed on the host (15 matmul-units of work -> 9).
  * All activations live on-chip in transposed layout ([feature, seq]) so no
    on-device transposes are needed anywhere:
        QT/KT from xT (host-transposed feaQK, zero-padded cols)
        scoresT[k,q] = KT chunks (stationary) x QT  (PSUM fp32)
        ET16 = 16*exp(scoresT/32 + mask)  (mask folded into exp bias)
        V0 (natural [s,d]) from host-transposed feaV as the stationary operand
        outU[q,d] = ET16 chunks (stationary) x V8; den[q] = ET16 x ones
        out = outU / den + (V0 + 2*vb)        [atten@vb == vb since sum(atten)=1]
  * fp8e4 DoubleRow matmuls (2 contraction blocks / instruction, ~1.5-1.8x bf16)
    for the Q-proj, K-stencil, scores and PV stages. Weights are scaled x256 on
    the host so they sit in fp8's normal range; Q/K are stored x16; ET x16.
    The V-projection stays bf16: out ~= V, so V's accuracy dominates the
    output and fp8 would blow the error budget. The PV matmul uses a separate
    fp8 copy (V8) of the unbiased projection.
  * Keys beyond seqlength are dead: K/scores/PV work only covers the first
    ceil(seqlen/128) key chunks per batch slot. Batches are paired
    longest-with-shortest across cores so the compile-time per-slot chunk
    counts (max over cores) stay small; sub-chunk masking still goes through
    the exp bias, so over-covering is always correct.
  * 16 batches -> 2 per core, full weights on every core. Output DMA in bf16.
"""

import numpy as np
import ml_dtypes

import concourse.bass as bass
from concourse import bacc
import concourse.tile as tile
from concourse import mybir

B, S, C, D = 16, 1024, 1024, 1024
P = 128
NCI, NDI, NKI, NQI, NSI = C // P, D // P, S // P, S // P, S // P
NF = 512  # matmul free dim (one PSUM bank of fp32)
PADL = 2   # left zero pad for the width-5 stencil
SP = 1040  # padded seq width; multiple of 16 so fp8 DoubleRow pair-stride is legal
LB = 2  # local batches per core
NCORES = 8
MASK_NEG = -60000.0
SCALE = 1.0 / 32.0  # 1/sqrt(D)
WS = 256.0   # host weight scale into fp8 normal range
AS = 16.0    # on-chip activation scale for QT/KT/ET
LOG_AS = float(np.log(AS))

BF = mybir.dt.bfloat16
F32 = mybir.dt.float32
F8 = mybir.dt.float8e4
AF = mybir.ActivationFunctionType
DR = mybir.MatmulPerfMode.DoubleRow

TRACE = False  # set by test harness to collect HW profile
_CACHE = {}
MARKS = []  # (label, first-instruction-name) per stage, for trace attribution


def _build_program(vs):
    nc = bacc.Bacc("TRN2", dynamic_dma_scratch_size=256)

    xt = nc.dram_tensor("xt", [LB, C, SP], F8, kind="ExternalInput")
    fvt = nc.dram_tensor("fvt", [LB, C, S], BF, kind="ExternalInput")
    wq = nc.dram_tensor("wq", [C, D], F8, kind="ExternalInput")
    wk = nc.dram_tensor("wk", [5, C, D], F8, kind="ExternalInput")
    wv = nc.dram_tensor("wv", [C, D], BF, kind="ExternalInput")
    qb = nc.dram_tensor("qb", [P, NDI], F32, kind="ExternalInput")
    kb = nc.dram_tensor("kb", [P, NDI], F32, kind="ExternalInput")
    vb = nc.dram_tensor("vb", [P, D], F32, kind="ExternalInput")
    mb = nc.dram_tensor("mb", [LB, P, NKI], F32, kind="ExternalInput")
    out = nc.dram_tensor("out", [LB, S, D], BF, kind="ExternalOutput")

    with tile.TileContext(nc) as tc:
        _emit(nc, tc, xt, fvt, wq, wk, wv, qb, kb, vb, mb, out, vs)
    nc.finalize()
    return nc


def _mark(nc, label):
    mx = 0
    for k in nc._state.inst_map:
        if k.startswith("I-"):
            try:
                mx = max(mx, int(k[2:].split("_")[0]))
            except ValueError:
                pass
    MARKS.append((label, mx))


def _emit(nc, tc, xt, fvt, wq, wk, wv, qb, kb, vb, mb, out, vs):
    from contextlib import ExitStack

    with ExitStack() as ctx:
        wpool = ctx.enter_context(tc.tile_pool(name="wpool", bufs=1))
        apool = ctx.enter_context(tc.tile_pool(name="apool", bufs=1))
        opool = ctx.enter_context(tc.tile_pool(name="opool", bufs=3))
        spool = ctx.enter_context(tc.tile_pool(name="spool", bufs=2))
        pp = ctx.enter_context(tc.tile_pool(name="pp", bufs=6, space="PSUM"))
        pd = ctx.enter_context(tc.tile_pool(name="pd", bufs=2, space="PSUM"))

        # Small constants first (cheap), then per-stage operands in the order
        # the PE consumes them, so the first matmul isn't stuck behind the
        # whole initial weight load.
        QB = wpool.tile([P, NDI], F32, tag="qb")
        nc.sync.dma_start(out=QB, in_=qb[:, :])
        KB = wpool.tile([P, NDI], F32, tag="kb")
        nc.sync.dma_start(out=KB, in_=kb[:, :])
        VB2 = wpool.tile([P, D], F32, tag="vb")
        nc.sync.dma_start(out=VB2, in_=vb[:, :])
        ONES = wpool.tile([P, 2, 16], F8, tag="ones")
        nc.vector.memset(ONES, 1.0)
        WV = wpool.tile([P, NCI, D], BF, tag="wv")
        WQ = wpool.tile([P, NCI, D], F8, tag="wq")
        WK = None

        for b in range(LB):
            v = vs[b]  # valid key chunks for this batch slot
            # key-dim psum groups: (offset, width) pieces covering v*128 cols
            kg = [(0, min(v * P, NF))]
            if v * P > NF:
                kg.append((NF, v * P - NF))

            # --- stage B: QT[d, s] = 16*Q (fp8 DoubleRow; smallest DMA lead-in)
            XT = apool.tile([P, NCI, SP], F8, tag="xt")
            for ci in range(NCI):
                nc.sync.dma_start(out=XT[:, ci, :], in_=xt[b, ci * P:(ci + 1) * P, :])
                if b == 0:
                    nc.sync.dma_start(out=WQ[:, ci, :], in_=wq[ci * P:(ci + 1) * P, :])
            MB = spool.tile([P, NKI], F32, tag="mb")
            nc.sync.dma_start(out=MB, in_=mb[b])
            _mark(nc, f"B{b}")
            QT = apool.tile([P, NDI, S], F8, tag="qt")
            for di in range(NDI):
                ps = [pp.tile([P, NF], F32, tag="ps", name=f"ps{_i}") for _i in range(2)]
                for cp in range(0, NCI, 2):
                    lhsT = WQ[:, cp:cp + 2, di * P:(di + 1) * P]
                    for sh in range(2):
                        nc.tensor.matmul(
                            ps[sh], lhsT,
                            XT[:, cp:cp + 2, PADL + sh * NF: PADL + sh * NF + NF],
                            start=(cp == 0), stop=(cp == NCI - 2), perf_mode=DR)
                for sh in range(2):
                    nc.scalar.activation(
                        QT[:, di, sh * NF:(sh + 1) * NF], ps[sh], AF.Identity,
                        bias=QB[:, di:di + 1], scale=AS / WS)

            # --- stage D: V0 natural [s, d]; bf16 Vb2 for +V, fp8 V8 for PV --
            FVT = apool.tile([P, NCI, S], BF, tag="fvt")
            for ci in range(NCI):
                nc.sync.dma_start(out=FVT[:, ci, :], in_=fvt[b, ci * P:(ci + 1) * P, :])
                if b == 0:
                    nc.sync.dma_start(out=WV[:, ci, :], in_=wv[ci * P:(ci + 1) * P, :])
            _mark(nc, f"D{b}")
            V = apool.tile([P, NSI, D], BF, tag="v")
            V8 = apool.tile([P, NSI, D], F8, tag="v8")
            for si in range(NSI):
                ps = [pp.tile([P, NF], F32, tag="ps", name=f"ps{_i}") for _i in range(2)]
                for ci in range(NCI):
                    lhsT = FVT[:, ci, si * P:(si + 1) * P]
                    for dh in range(2):
                        nc.tensor.matmul(
                            ps[dh], lhsT, WV[:, ci, dh * NF:(dh + 1) * NF],
                            start=(ci == 0), stop=(ci == NCI - 1))
                for dh in range(2):
                    nc.vector.tensor_add(
                        V[:, si, dh * NF:(dh + 1) * NF], ps[dh],
                        VB2[:, dh * NF:(dh + 1) * NF])
                    if si < v:
                        nc.vector.tensor_copy(
                            V8[:, si, dh * NF:(dh + 1) * NF], ps[dh])

            # --- stage C: KT[d, s] = 16*K (width-5 stencil, only v key chunks)
            if WK is None:
                WK = []
                for j in range(5):
                    t = wpool.tile([P, NCI, D], F8, tag=f"wk{j}")
                    for ci in range(NCI):
                        nc.sync.dma_start(
                            out=t[:, ci, :], in_=wk[j, ci * P:(ci + 1) * P, :])
                    WK.append(t)
            _mark(nc, f"C{b}")
            KT = apool.tile([P, NDI, S], F8, tag="kt")
            for di in range(NDI):
                ps = [pp.tile([P, NF], F32, tag="ps", name=f"ps{_i}")
                      for _i in range(len(kg))]
                step, nsteps = 0, 5 * (NCI // 2)
                for j in range(5):
                    for cp in range(0, NCI, 2):
                        lhsT = WK[j][:, cp:cp + 2, di * P:(di + 1) * P]
                        for g, (off, w) in enumerate(kg):
                            nc.tensor.matmul(
                                ps[g][:, :w], lhsT,
                                XT[:, cp:cp + 2, j + off: j + off + w],
                                start=(step == 0), stop=(step == nsteps - 1),
                                perf_mode=DR)
                        step += 1
                for g, (off, w) in enumerate(kg):
                    nc.scalar.activation(
                        KT[:, di, off:off + w], ps[g][:, :w], AF.Identity,
                        bias=KB[:, di:di + 1], scale=AS / WS)

            # --- stage E: ET16[k, q] = 16*exp(scoresT/32 + mask) -------------
            _mark(nc, f"E{b}")
            ET = apool.tile([P, NKI, S], F8, tag="et")
            for ki in range(v):
                ps = [pp.tile([P, NF], F32, tag="ps", name=f"ps{_i}") for _i in range(2)]
                for dp in range(0, NDI, 2):
                    lhsT = KT[:, dp:dp + 2, ki * P:(ki + 1) * P]
                    for qh in range(2):
                        nc.tensor.matmul(
                            ps[qh], lhsT, QT[:, dp:dp + 2, qh * NF:(qh + 1) * NF],
                            start=(dp == 0), stop=(dp == NDI - 2), perf_mode=DR)
                for qh in range(2):
                    nc.scalar.activation(
                        ET[:, ki, qh * NF:(qh + 1) * NF], ps[qh], AF.Exp,
                        bias=MB[:, ki:ki + 1], scale=SCALE / (AS * AS))

            # --- stage F: out = (ET16^T @ V8) / den + Vb2 --------------------
            _mark(nc, f"F{b}")
            for qi in range(NQI):
                pso = [pp.tile([P, NF], F32, tag="ps", name=f"pso{_i}") for _i in range(2)]
                psd = pd.tile([P, 1], F32, tag="den")
                for kp in range(0, v - (v % 2), 2):
                    lhsT = ET[:, kp:kp + 2, qi * P:(qi + 1) * P]
                    st, sp_ = (kp == 0), (kp + 2 >= v)
                    for dh in range(2):
                        nc.tensor.matmul(
                            pso[dh], lhsT, V8[:, kp:kp + 2, dh * NF:(dh + 1) * NF],
                            start=st, stop=sp_, perf_mode=DR)
                    nc.tensor.matmul(psd, lhsT, ONES[:, :, 0:1],
                                     start=st, stop=sp_, perf_mode=DR)
                if v % 2:
                    ki = v - 1
                    lhsT = ET[:, ki, qi * P:(qi + 1) * P]
                    st = (v == 1)
                    for dh in range(2):
                        nc.tensor.matmul(
                            pso[dh], lhsT, V8[:, ki, dh * NF:(dh + 1) * NF],
                            start=st, stop=True)
                    nc.tensor.matmul(psd, lhsT, ONES[:, 0, 0:1],
                                     start=st, stop=True)
                REC = spool.tile([P, 1], F32, tag="rec")
                nc.vector.reciprocal(REC, psd)
                for dh in range(2):
                    OT = opool.tile([P, NF], F32, tag="otf", name=f"ot{dh}")
                    nc.scalar.activation(OT, pso[dh], AF.Copy, bias=0.0, scale=REC)
                    OB = opool.tile([P, NF], BF, tag="obf", name=f"ob{dh}")
                    nc.vector.tensor_add(
                        OB, OT, V[:, qi, dh * NF:(dh + 1) * NF])
                    nc.sync.dma_start(
                        out=out[b, qi * P:(qi + 1) * P, dh * NF:(dh + 1) * NF],
                        in_=OB)


def _prep_host(feaQK, feaV, seqlengths, cn3_w, cn3_b, cn5_w, cn5_b,
               k_w, k_b, q_w, q_b, v_w, v_b):
    """Compose weights, assign batches to cores, lay out per-core inputs."""
    f32 = np.float32
    bf16 = ml_dtypes.bfloat16
    f8 = ml_dtypes.float8_e4m3
    feaQK = np.asarray(feaQK, f32)
    feaV = np.asarray(feaV, f32)
    seqlengths = np.asarray(seqlengths).astype(np.int64)

    W1 = np.asarray(k_w, f32)[:, :C]
    W2 = np.asarray(k_w, f32)[:, C:2 * C]
    W3 = np.asarray(k_w, f32)[:, 2 * C:]

    wk = np.zeros((5, C, D), f32)  # [tap j (= shift+2), c, d]
    for t in range(3):
        wk[t + 1] += (W2 @ np.asarray(cn3_w, f32)[:, :, t]).T
    for t in range(5):
        wk[t] += (W3 @ np.asarray(cn5_w, f32)[:, :, t]).T
    wk[2] += W1.T
    kb_eff = (np.asarray(k_b, f32) + W2 @ np.asarray(cn3_b, f32)
              + W3 @ np.asarray(cn5_b, f32))

    wq = np.ascontiguousarray(np.asarray(q_w, f32).T)
    wv = np.ascontiguousarray(np.asarray(v_w, f32).T)

    qb_pd = np.ascontiguousarray((np.asarray(q_b, f32) * AS).reshape(NDI, P).T)
    kb_pd = np.ascontiguousarray((kb_eff * AS).reshape(NDI, P).T)
    vb2_rep = np.ascontiguousarray(
        np.broadcast_to(2.0 * np.asarray(v_b, f32), (P, D)))

    key_valid = np.arange(S)[None, :] < seqlengths[:, None]
    mask = np.where(key_valid, LOG_AS, MASK_NEG).astype(f32)  # [B, S]

    # Pair longest with shortest so the compile-time per-slot chunk counts
    # (max over cores) stay near the per-core optimum.
    vchunks = np.clip(np.ceil(seqlengths / P).astype(int), 1, NKI)
    order = np.argsort(-seqlengths, kind="stable")
    batch_of = np.zeros((NCORES, LB), int)
    for i in range(NCORES):
        batch_of[i, 0] = order[B - 1 - i]
        batch_of[i, 1] = order[i]
    vs = (int(vchunks[batch_of[:, 0]].max()),
          int(vchunks[batch_of[:, 1]].max()))

    wq_8 = np.clip(wq * WS, -240, 240).astype(f8)
    wk_8 = np.ascontiguousarray(np.clip(wk * WS, -240, 240).astype(f8))
    wv_b = wv.astype(bf16)

    in_maps = []
    for core in range(NCORES):
        bs = batch_of[core]
        xts = np.zeros((LB, C, SP), f8)
        xts[:, :, PADL:PADL + S] = np.clip(
            feaQK[bs].transpose(0, 2, 1), -240, 240).astype(f8)
        fvts = np.ascontiguousarray(
            feaV[bs].transpose(0, 2, 1)).astype(bf16)
        mbs = np.ascontiguousarray(
            mask[bs].reshape(LB, NKI, P).transpose(0, 2, 1))
        in_maps.append({
            "xt": xts, "fvt": fvts,
            "wq": wq_8, "wk": wk_8, "wv": wv_b,
            "qb": qb_pd, "kb": kb_pd, "vb": vb2_rep, "mb": mbs,
        })
    return in_maps, batch_of, vs


def kernel(**inputs):
    from concourse.bass_utils import run_bass_kernel_spmd

    in_maps, batch_of, vs = _prep_host(**inputs)
    if _CACHE.get("vs") != vs:
        _CACHE["nc"] = _build_program(vs)
        _CACHE["vs"] = vs
    nc = _CACHE["nc"]
    res = run_bass_kernel_spmd(nc, in_maps, core_ids=list(range(NCORES)),
                               trace=TRACE)
    _CACHE["last_result"] = res
    full = np.zeros((B, S, D), np.float32)
    for core in range(NCORES):
        full[batch_of[core]] = res.results[core]["out"].astype(np.float32)
    return full
